# revision 29
# baseline (speedup 1.0000x reference)
"""Trainium2 Bass kernel for a single transformer decoder layer.

Sharding: 8 cores = 4 batches x 2 head-groups (tensor parallel over heads for
attention; pairwise ReduceScatter; token-split FFN). All activations are kept
feature-major ("transposed") on device; the host transposes inputs/outputs.
Token ownership after the reduce-scatter is by interleaved 512-blocks
({0,2} / {1,3}).

v2: full bf16 datapath (weights, activations, collective payloads), softmax
scale folded into Wq on the host, 2-bank PSUM score tiles with one wide exp,
approx reciprocals, normalize-broadcast via a tiny PE matmul, W1 resident in
SBUF so the FFN overlaps the second reduce-scatter.
"""

import contextlib
import sys

for _p in ("/opt/trn_rl_repo",):
    if _p not in sys.path:
        sys.path.insert(0, _p)

import numpy as np

import concourse.bass as bass
import concourse.mybir as mybir
import concourse.tile as tile
from concourse import bacc
from concourse.bass_utils import run_bass_kernel_spmd

# ---- problem constants (hardcoded per spec) ----
B, S, D = 4, 2048, 1024
H, DK, DV, DFF = 16, 64, 64, 4096
EPS = 1e-5
SCALE = 1.0 / 32.0  # 1/sqrt(D), folded into Wq on the host

NCORES = 8
HL = H // 2          # heads per core (local)
NP = HL // 2         # head-pairs per core (4)
TLOC = S // 2        # tokens owned per core after reduce-scatter (1024)
DC = D // 128        # d-model chunks (8)
FC = DFF // 128      # dff chunks (32)
FQ = FC // 4         # dff chunks per quarter (8)
QB = S // 512        # query blocks of 512 (4)

F32 = mybir.dt.float32
F32R = mybir.dt.float32r
BF16 = mybir.dt.bfloat16

DEBUG = False
_COMPILED = None


def _build():
    nc = bacc.Bacc("TRN2", target_bir_lowering=False, debug=False,
                   num_devices=NCORES)

    xT_d = nc.dram_tensor("xT", [128, DC, S], BF16, kind="ExternalInput").ap()
    xTm_d = nc.dram_tensor("xTm", [128, DC, TLOC], BF16,
                           kind="ExternalInput").ap()
    wq_d = nc.dram_tensor("wq", [128, NP, DC, 128], BF16,
                          kind="ExternalInput").ap()
    wk_d = nc.dram_tensor("wk", [128, NP, DC, 128], BF16,
                          kind="ExternalInput").ap()
    wv_d = nc.dram_tensor("wv", [128, DC, 512], BF16,
                          kind="ExternalInput").ap()
    wo_d = nc.dram_tensor("wo", [128, NP, DC, 128], BF16,
                          kind="ExternalInput").ap()
    w1_d = nc.dram_tensor("w1", [128, FC, DC, 128], BF16,
                          kind="ExternalInput").ap()
    w2_d = nc.dram_tensor("w2", [128, 4, 4, FQ, 2, 128], BF16,
                          kind="ExternalInput").ap()
    b1_d = nc.dram_tensor("b1s", [128, FC], F32, kind="ExternalInput").ap()
    b2_d = nc.dram_tensor("b2s", [128, DC], F32, kind="ExternalInput").ap()
    g1_d = nc.dram_tensor("g1s", [128, DC], F32, kind="ExternalInput").ap()
    e1_d = nc.dram_tensor("e1s", [128, DC], F32, kind="ExternalInput").ap()
    g2_d = nc.dram_tensor("g2s", [128, DC], F32, kind="ExternalInput").ap()
    e2_d = nc.dram_tensor("e2s", [128, DC], F32, kind="ExternalInput").ap()
    mk_d = nc.dram_tensor("mask", [128, 4, 512], BF16,
                          kind="ExternalInput").ap()

    outT_d = nc.dram_tensor("outT", [128, DC, TLOC], F32,
                            kind="ExternalOutput").ap()
    dbg = None
    if DEBUG:
        dbg = {
            "dbg_qt": nc.dram_tensor("dbg_qt", [128, NP, S], BF16,
                                     kind="ExternalOutput").ap(),
            "dbg_kt": nc.dram_tensor("dbg_kt", [128, NP, S], BF16,
                                     kind="ExternalOutput").ap(),
            "dbg_v": nc.dram_tensor("dbg_v", [128, S // 128, HL * 65], BF16,
                                    kind="ExternalOutput").ap(),
            "dbg_eab": nc.dram_tensor("dbg_eab", [128, 2, 512], BF16,
                                      kind="ExternalOutput").ap(),
            "dbg_ctx": nc.dram_tensor("dbg_ctx", [128, NP, 512], BF16,
                                      kind="ExternalOutput").ap(),
            "dbg_rsin": nc.dram_tensor("dbg_rsin", [2, D, 512], BF16,
                                       kind="ExternalOutput").ap(),
            "dbg_rso": nc.dram_tensor("dbg_rso", [D, 512], BF16,
                                      kind="ExternalOutput").ap(),
            "dbg_h1": nc.dram_tensor("dbg_h1", [128, DC, 512], BF16,
                                     kind="ExternalOutput").ap(),
            "dbg_o2": nc.dram_tensor("dbg_o2", [128, DC, 512], F32,
                                     kind="ExternalOutput").ap(),
        }

    with tile.TileContext(nc) as tc:
        _emit(nc, tc, xT_d, xTm_d, wq_d, wk_d, wv_d, wo_d, w1_d, w2_d,
              b1_d, b2_d, g1_d, e1_d, g2_d, e2_d, mk_d, outT_d, dbg)
    nc.compile()
    return nc


def _emit(nc, tc, xT_d, xTm_d, wq_d, wk_d, wv_d, wo_d, w1_d, w2_d,
          b1_d, b2_d, g1_d, e1_d, g2_d, e2_d, mk_d, outT_d, dbg=None):
    AF = mybir.ActivationFunctionType
    ALU = mybir.AluOpType

    with (
        tc.tile_pool(name="dram", bufs=1, space="DRAM") as dram,
        tc.tile_pool(name="const", bufs=1) as const,
        tc.tile_pool(name="pStA", bufs=2) as pStA,     # mu/rstd, 8KB
        tc.tile_pool(name="pStB", bufs=1) as pStB,     # var/srt, 4KB
        tc.tile_pool(name="pBC", bufs=2) as pBC,       # LN broadcasts, 4KB
        tc.tile_pool(name="pH1", bufs=1) as pH1,       # h1 bf16, 16KB
        tc.tile_pool(name="pDW", bufs=1) as pDW,       # d_half work, 24KB
    ):
        MASK = const.tile([128, 4, 512], BF16)
        # ones (value 1/D) for LN mean/var matmuls, f32r + bf16 flavors
        onesf = const.tile([128, 1], F32)
        nc.vector.memset(onesf[:], 1.0 / D)
        onesr = const.tile([128, 1], F32R)
        nc.vector.tensor_copy(onesr[:], onesf[:])
        onesb = const.tile([128, 1], BF16)
        nc.vector.tensor_copy(onesb[:], onesf[:])
        one1 = const.tile([128, 1], BF16)
        nc.vector.memset(one1[:], 1.0)
        # broadcast helper: ones64.T @ rec[1,512] -> [64,512]
        ones64 = const.tile([1, 64], BF16)
        nc.vector.memset(ones64[:], 1.0)
        epst = const.tile([1, 1], F32)
        nc.vector.memset(epst[:], EPS)
        g1t = const.tile([128, DC], F32)
        e1t = const.tile([128, DC], F32)
        g2t = const.tile([128, DC], F32)
        e2t = const.tile([128, DC], F32)
        b1t = const.tile([128, FC], F32)
        b2t = const.tile([128, DC], F32)

        def load_consts():
            nc.sync.dma_start(MASK[:], mk_d[:])
            for t_, d_ in ((g1t, g1_d), (e1t, e1_d), (g2t, g2_d),
                           (e2t, e2_d), (b1t, b1_d), (b2t, b2_d)):
                nc.sync.dma_start(t_[:], d_[:])

        rs_in0 = dram.tile([2, D, 512], BF16)
        rs_in1 = dram.tile([2, D, 512], BF16)
        rs_out0 = dram.tile([D, 512], BF16)
        rs_out1 = dram.tile([D, 512], BF16)

        def layer_norm_finish_stats(stat):
            """stat rows 0/64 hold sum*1/D and sumsq*1/D; -> (mu, rstd)."""
            mu = pStA.tile([1, 512], F32, tag="lnmu")
            nc.vector.tensor_copy(mu[:], stat[0:1, :])
            var = pStB.tile([1, 512], F32, tag="lnvar")
            nc.vector.tensor_mul(var[:], mu[:], mu[:])
            nc.vector.tensor_sub(var[:], stat[64:65, :], var[:])
            srt = pStB.tile([1, 512], F32, tag="lnsrt")
            nc.scalar.activation(srt[:], var[:], AF.Sqrt, bias=epst[:])
            rstd = pStA.tile([1, 512], F32, tag="lnrstd")
            nc.vector.reciprocal_approx_fast(rstd[:], srt[:])
            return mu, rstd

        def layer_norm_stats_f32r(src, sqb, psLN):
            """src: [128, DC, 512] f32r AP (properly rounded)."""
            nc.scalar.activation(sqb[:], src, AF.Square)
            stat = psLN.tile([128, 512], F32, tag="ctx")
            for dc in range(DC):
                nc.tensor.matmul(stat[0:1, :], onesr[:], src[:, dc],
                                 start=(dc == 0), stop=(dc == DC - 1))
            for dc in range(DC):
                nc.tensor.matmul(stat[64:65, :], onesb[:], sqb[:, dc],
                                 start=(dc == 0), stop=(dc == DC - 1))
            return layer_norm_finish_stats(stat)

        def layer_norm_apply(src, dst, mu, rstd, gt, et):
            """dst[:,dc] = ((src - mu) * rstd) * g + e; src modified in
            place. Broadcasts on gpsimd (safe: only used after the relevant
            collective in gpsimd queue order)."""
            mub = pBC.tile([128, 512], F32, tag="lnb")
            rstdb = pBC.tile([128, 512], F32, tag="lnb")
            nc.gpsimd.partition_broadcast(mub[:], mu[:])
            nc.gpsimd.partition_broadcast(rstdb[:], rstd[:])
            nc.vector.tensor_sub(
                src, src, mub[:, None, :].to_broadcast((128, DC, 512)))
            nc.vector.tensor_mul(
                src, src, rstdb[:, None, :].to_broadcast((128, DC, 512)))
            for dc in range(DC):
                nc.vector.tensor_scalar(
                    dst[:, dc], src[:, dc], gt[:, dc:dc + 1],
                    et[:, dc:dc + 1], ALU.mult, ALU.add)

        H1b = [pH1.tile([128, DC, 512], BF16, tag=f"H1b{h}",
                        name=f"H1b{h}") for h in range(2)]

        def d_half_dma(h, rso):
            """Kick off the DMAs for h1 = LN1(x + attn_out), half h."""
            aor = pDW.tile([128, DC, 512], BF16, tag="aor", name=f"aor{h}")
            nc.gpsimd.dma_start(
                aor[:], rso.rearrange("(dc p) t -> p dc t", p=128))
            xm = pDW.tile([128, DC, 512], BF16, tag="xm", name=f"xm{h}")
            nc.sync.dma_start(xm[:], xTm_d[:, :, h * 512:(h + 1) * 512])
            return aor, xm

        def d_half_prep(h, aor, xm, eng):
            """Elementwise prep; emit where `eng`'s queue has slack."""
            eng.tensor_add(xm[:], xm[:], aor[:])
            sq = pDW.tile([128, DC, 512], BF16, tag="sq", name=f"sq{h}")
            eng.tensor_mul(sq[:], xm[:], xm[:])
            return xm, xm, sq

        def d_half_finish(h, state, psLN, eng):
            """Stat matmuls + LN apply; emit where the PE has slack."""
            xm, xb, sq = state
            stat = psLN.tile([128, 512], F32, tag="ctx")
            for dc in range(DC):
                nc.tensor.matmul(stat[0:1, :], onesb[:], xb[:, dc],
                                 start=(dc == 0), stop=(dc == DC - 1))
            for dc in range(DC):
                nc.tensor.matmul(stat[64:65, :], onesb[:], sq[:, dc],
                                 start=(dc == 0), stop=(dc == DC - 1))
            mu, rstd = layer_norm_finish_stats(stat)
            mub = pBC.tile([128, 512], F32, tag="lnb")
            rstdb = pBC.tile([128, 512], F32, tag="lnb")
            nc.gpsimd.partition_broadcast(mub[:], mu[:])
            nc.gpsimd.partition_broadcast(rstdb[:], rstd[:])
            eng.tensor_sub(
                xm[:], xm[:], mub[:, None, :].to_broadcast((128, DC, 512)))
            eng.tensor_mul(
                xm[:], xm[:], rstdb[:, None, :].to_broadcast((128, DC, 512)))
            for dc in range(DC):
                eng.tensor_scalar(
                    H1b[h][:, dc], xm[:, dc], g1t[:, dc:dc + 1],
                    e1t[:, dc:dc + 1], ALU.mult, ALU.add)

        w1stack = contextlib.ExitStack()
        with (
            tc.tile_pool(name="pQKT", bufs=1) as pQKT,
            tc.tile_pool(name="pV", bufs=1) as pV,
        ):
            QT = pQKT.tile([128, NP, S], BF16, tag="QT")
            KT = pQKT.tile([128, NP, S], BF16, tag="KT")
            V = pV.tile([128, S // 128, HL * 65], BF16, tag="V")
            nc.vector.tensor_copy(
                V[:].rearrange("p t (h c) -> p t h c", c=65)[:, :, :, 64:65],
                one1[:, None, None, :].to_broadcast((128, S // 128, HL, 1)))

            # ================= Q/K/V projections =================
            with (
                tc.tile_pool(name="pX", bufs=1) as pX,
                tc.tile_pool(name="pWqkv", bufs=1) as pWqkv,
            ):
                wqt = pWqkv.tile([128, NP, DC, 128], BF16, tag="wq")
                wkt = pWqkv.tile([128, NP, DC, 128], BF16, tag="wk")
                wvt = pWqkv.tile([128, DC, 512], BF16, tag="wv")
                nc.sync.dma_start(wqt[:], wq_d[:])
                nc.sync.dma_start(wkt[:], wk_d[:])
                X = pX.tile([128, DC, S], BF16, tag="X")
                for dc in range(DC):
                    nc.sync.dma_start(X[:, dc], xT_d[:, dc])
                nc.sync.dma_start(wvt[:], wv_d[:])
                load_consts()

                with tc.tile_pool(name="psP", bufs=8, space="PSUM") as psP:
                    for p in range(NP):
                        pqs = [psP.tile([128, 512], F32, tag="proj",
                                        name=f"pq_{p}_{i}") for i in range(8)]
                        for dc in range(DC):
                            for tb in range(QB):
                                nc.tensor.matmul(
                                    pqs[tb][:], wqt[:, p, dc],
                                    X[:, dc, tb * 512:(tb + 1) * 512],
                                    start=(dc == 0), stop=(dc == DC - 1))
                            for tb in range(QB):
                                nc.tensor.matmul(
                                    pqs[4 + tb][:], wkt[:, p, dc],
                                    X[:, dc, tb * 512:(tb + 1) * 512],
                                    start=(dc == 0), stop=(dc == DC - 1))
                        for tb in range(QB):
                            tsl = slice(tb * 512, (tb + 1) * 512)
                            nc.scalar.activation(QT[:, p, tsl], pqs[tb][:],
                                                 AF.Identity)
                            nc.vector.tensor_copy(KT[:, p, tsl],
                                                  pqs[4 + tb][:])

                with tc.tile_pool(name="psV", bufs=3, space="PSUM") as psV:
                    for tt in range(S // 128):
                        pv = psV.tile([128, 512], F32, tag="pv")
                        for dc in range(DC):
                            nc.tensor.matmul(
                                pv[:], X[:, dc, tt * 128:(tt + 1) * 128],
                                wvt[:, dc],
                                start=(dc == 0), stop=(dc == DC - 1))
                        nc.vector.tensor_copy(
                            V[:, tt].rearrange("p (h c) -> p h c",
                                               c=65)[:, :, 0:64],
                            pv[:].rearrange("p (h c) -> p h c", c=64))

            if dbg is not None:
                nc.sync.dma_start(dbg["dbg_qt"][:], QT[:])
                nc.sync.dma_start(dbg["dbg_kt"][:], KT[:])
                nc.sync.dma_start(dbg["dbg_v"][:], V[:])

            # W1 stays resident through the FFN (ExitStack outlives this
            # block); loaded during attention via the freed X/Wqkv space.
            pW1 = w1stack.enter_context(tc.tile_pool(name="pW1", bufs=1,
                                                     side="right"))
            w1t = pW1.tile([128, FC, DC, 128], BF16, tag="w1")
            for fq in range(4):
                nc.sync.dma_start(w1t[:, fq * FQ:(fq + 1) * FQ],
                                  w1_d[:, fq * FQ:(fq + 1) * FQ])

            # ==================== attention ====================
            with (
                tc.tile_pool(name="pWO", bufs=1) as pWO,
                tc.tile_pool(name="pCTX", bufs=1) as pCTX,
                tc.tile_pool(name="pE", bufs=2) as pE,
                tc.tile_pool(name="pAO", bufs=2) as pAO,
                tc.tile_pool(name="pRec", bufs=1) as pRec,
                tc.tile_pool(name="psS", bufs=3, space="PSUM") as psS,
                tc.tile_pool(name="psC", bufs=2, space="PSUM") as psC,
            ):
                wot = pWO.tile([128, NP, DC, 128], BF16, tag="wo")
                nc.sync.dma_start(wot[:], wo_d[:])

                pend = [None]
                d0_dma = [None]
                d0_state = [None]

                def flush_norm():
                    if pend[0] is None:
                        return
                    ctx_t, pp, rec16 = pend[0]
                    pend[0] = None
                    rb = psS.tile([128, 2, 512], F32, tag="sc")
                    recb = rb[:, 0]
                    nc.tensor.matmul(recb[0:64, :], ones64[:],
                                     rec16[0:1, 0:512],
                                     start=True, stop=True)
                    nc.tensor.matmul(recb[64:128, :], ones64[:],
                                     rec16[0:1, 512:1024],
                                     start=True, stop=True)
                    nc.vector.tensor_mul(ctx_t[:, pp, :], ctx_t[:, pp, :],
                                         recb[:])

                for qb in range(QB):
                    qsl = slice(qb * 512, (qb + 1) * 512)
                    nkc = 4 * (qb + 1)
                    CTX = pCTX.tile([128, NP, 512], BF16, tag="CTX")
                    for p in range(NP):
                        ctxA = psC.tile([128, 512], F32, tag="ctx")
                        ctxB = psC.tile([128, 512], F32, tag="ctx")
                        for kc in range(nkc):
                            ksl = slice(kc * 128, (kc + 1) * 128)
                            ps = psS.tile([128, 2, 512], F32, tag="sc")
                            nc.tensor.matmul(ps[:, 0], KT[0:64, p, ksl],
                                             QT[0:64, p, qsl],
                                             start=True, stop=True)
                            nc.tensor.matmul(ps[:, 1], KT[64:128, p, ksl],
                                             QT[64:128, p, qsl],
                                             start=True, stop=True)
                            eAB = pE.tile([128, 2, 512], BF16, tag="E")
                            nc.scalar.activation(eAB[:], ps[:], AF.Exp)
                            if kc >= 4 * qb:
                                mkc = kc - 4 * qb
                                nc.vector.tensor_mul(
                                    eAB[:], eAB[:],
                                    MASK[:, mkc][:, None, :]
                                    .to_broadcast((128, 2, 512)))
                            if dbg is not None and qb == 0 and p == 0 \
                                    and kc == 0:
                                nc.sync.dma_start(dbg["dbg_eab"][:], eAB[:])
                            st, sp = (kc == 0), (kc == nkc - 1)
                            nc.tensor.matmul(
                                ctxA[0:65, :],
                                V[:, kc, 2 * p * 65:(2 * p + 1) * 65],
                                eAB[:, 0], start=st, stop=sp)
                            nc.tensor.matmul(
                                ctxB[0:65, :],
                                V[:, kc, (2 * p + 1) * 65:(2 * p + 2) * 65],
                                eAB[:, 1], start=st, stop=sp)
                        # normalize: rec = 1/rowsum; per-head PE broadcast
                        den = pRec.tile([1, 1024], F32, tag="den")
                        nc.vector.tensor_copy(den[0:1, 0:512],
                                              ctxA[64:65, :])
                        nc.vector.tensor_copy(den[0:1, 512:1024],
                                              ctxB[64:65, :])
                        recF = pRec.tile([1, 1024], F32, tag="recF")
                        rec16 = pRec.tile([1, 1024], BF16, tag="rec16")
                        nc.vector.reciprocal_approx_fast(recF[:], den[:])
                        nc.vector.tensor_copy(rec16[:], recF[:])
                        nc.vector.tensor_copy(CTX[0:64, p, :], ctxA[0:64, :])
                        nc.vector.tensor_copy(CTX[64:128, p, :],
                                              ctxB[0:64, :])
                        flush_norm()
                        pend[0] = (CTX, p, rec16)
                        if qb == 3 and p == 0:
                            d0_state[0] = d_half_prep(0, *d0_dma[0],
                                                      nc.vector)
                        if qb == 3 and p == 1:
                            d_half_finish(0, d0_state[0], psC, nc.vector)
                    flush_norm()
                    if dbg is not None and qb == 0:
                        nc.sync.dma_start(dbg["dbg_ctx"][:], CTX[:])
                    # Wo partial for this token block
                    rsdst = rs_in0 if qb < 2 else rs_in1
                    for dout in range(DC):
                        po = psC.tile([128, 512], F32, tag="ctx")
                        for p in range(NP):
                            nc.tensor.matmul(po[:], wot[:, p, dout],
                                             CTX[:, p, :],
                                             start=(p == 0),
                                             stop=(p == NP - 1))
                        ao = pAO.tile([128, 512], BF16, tag="ao")
                        nc.vector.tensor_copy(ao[:], po[:])
                        nc.sync.dma_start(
                            rsdst[qb % 2, dout * 128:(dout + 1) * 128, :],
                            ao[:])
                    if qb == 1:
                        nc.gpsimd.collective_compute(
                            "ReduceScatter", mybir.AluOpType.add,
                            replica_groups=[[0, 1], [2, 3], [4, 5], [6, 7]],
                            ins=[rs_in0.opt()], outs=[rs_out0.opt()])
                        if dbg is not None:
                            nc.gpsimd.dma_start(dbg["dbg_rsin"][:],
                                                rs_in0[:])
                            nc.gpsimd.dma_start(dbg["dbg_rso"][:],
                                                rs_out0[:])
                        d0_dma[0] = d_half_dma(0, rs_out0)

                nc.gpsimd.collective_compute(
                    "ReduceScatter", mybir.AluOpType.add,
                    replica_groups=[[0, 1], [2, 3], [4, 5], [6, 7]],
                    ins=[rs_in1.opt()], outs=[rs_out1.opt()])

        # ======== FFN (W1 resident, W2 streamed) ========
        with (
            tc.tile_pool(name="pFF", bufs=1) as pFF,
            tc.tile_pool(name="pO2", bufs=1) as pO2,
            tc.tile_pool(name="pW2q", bufs=3) as pW2q,
            tc.tile_pool(name="pOT", bufs=2) as pOT,
            tc.tile_pool(name="psF", bufs=2, space="PSUM") as psF,
            tc.tile_pool(name="psG", bufs=4, space="PSUM") as psG,
            tc.tile_pool(name="psL", bufs=2, space="PSUM") as psL,
        ):
            FFt = pFF.tile([128, FQ, 512], BF16, tag="FF")
            O2h = [pO2.tile([128, DC, 512], F32R, tag=f"O2_{h}",
                            name=f"O2_{h}") for h in range(2)]

            def ffn_half(th, hooks=None):
                for fq in range(4):
                    if hooks and fq in hooks:
                        hooks[fq]()
                    for fi in range(FQ):
                        fc = fq * FQ + fi
                        pf = psF.tile([128, 512], F32, tag="ff")
                        for dc in range(DC):
                            nc.tensor.matmul(
                                pf[:], w1t[:, fc, dc], H1b[th][:, dc],
                                start=(dc == 0), stop=(dc == DC - 1))
                        nc.scalar.activation(FFt[:, fi], pf[:], AF.Relu,
                                             bias=b1t[:, fc:fc + 1])
                    if hooks and ("w1_" + str(fq)) in hooks:
                        hooks["w1_" + str(fq)]()
                    for dq in range(4):
                        w2q = pW2q.tile([128, FQ, 2, 128], BF16, tag="w2")
                        nc.sync.dma_start(w2q[:], w2_d[:, fq, dq])
                        pos = [psG.tile([128, 512], F32, tag="o2",
                                        name=f"o2_{i}") for i in range(2)]
                        for fi in range(FQ):
                            for do2 in range(2):
                                nc.tensor.matmul(
                                    pos[do2][:], w2q[:, fi, do2], FFt[:, fi],
                                    start=(fi == 0), stop=(fi == FQ - 1))
                        for do2 in range(2):
                            dout = dq * 2 + do2
                            if fq == 0:
                                # O2 = W2 partial + b2 (bias folded in here)
                                nc.vector.tensor_scalar_add(
                                    O2h[th][:, dout], pos[do2][:],
                                    b2t[:, dout:dout + 1])
                            else:
                                nc.vector.tensor_add(
                                    O2h[th][:, dout], O2h[th][:, dout],
                                    pos[do2][:])
                            if fq == 3:
                                # final quarter: fold in the h1 residual
                                nc.vector.tensor_add(
                                    O2h[th][:, dout], O2h[th][:, dout],
                                    H1b[th][:, dout])

            def finish_half(th):
                # O2h already holds ff + h1 + b2; LN2 + store, per-dc
                if dbg is not None and th == 0:
                    nc.sync.dma_start(dbg["dbg_h1"][:], H1b[0][:])
                    nc.sync.dma_start(dbg["dbg_o2"][:],
                                      O2h[0][:].bitcast(F32))
                sqb = pFF.tile([128, DC, 512], BF16, tag="fsq",
                               name=f"fsq{th}")
                mu, rstd = layer_norm_stats_f32r(O2h[th][:], sqb, psL)
                mub = pBC.tile([128, 512], F32, tag="lnb")
                rstdb = pBC.tile([128, 512], F32, tag="lnb")
                nc.gpsimd.partition_broadcast(mub[:], mu[:])
                nc.gpsimd.partition_broadcast(rstdb[:], rstd[:])
                for dc in range(DC):
                    nc.vector.tensor_sub(O2h[th][:, dc], O2h[th][:, dc],
                                         mub[:])
                    nc.vector.tensor_mul(O2h[th][:, dc], O2h[th][:, dc],
                                         rstdb[:])
                    ot = pOT.tile([128, 512], F32, tag="ot")
                    nc.vector.tensor_scalar(
                        ot[:], O2h[th][:, dc], g2t[:, dc:dc + 1],
                        e2t[:, dc:dc + 1], ALU.mult, ALU.add)
                    nc.sync.dma_start(
                        outT_d[:, dc, th * 512:(th + 1) * 512], ot[:])

            d1_dma = d_half_dma(1, rs_out1)
            d1_state = [None]

            def d1_prep():
                d1_state[0] = d_half_prep(1, *d1_dma, nc.vector)

            def d1_finish():
                d_half_finish(1, d1_state[0], psL, nc.vector)

            ffn_half(0, hooks={2: d1_prep, "w1_3": d1_finish})
            finish_half(0)
            ffn_half(1)
            finish_half(1)
        w1stack.close()


def _pack_inputs(x, Wq, Wk, Wv, Wo, ln1_g, ln1_b, W1, b1, W2, b2, ln2_g,
                 ln2_b):
    """Build the 8 per-core input maps (all host-side numpy)."""
    import ml_dtypes
    bf = ml_dtypes.bfloat16
    f = np.float32
    x = np.asarray(x, f)
    Wq = np.asarray(Wq, f); Wk = np.asarray(Wk, f); Wv = np.asarray(Wv, f)
    Wo = np.asarray(Wo, f); W1 = np.asarray(W1, f); W2 = np.asarray(W2, f)

    w1p = np.ascontiguousarray(
        W1.reshape(DC, 128, FC, 128).transpose(1, 2, 0, 3)).astype(bf)
    w2p = np.ascontiguousarray(
        W2.reshape(4, FQ, 128, 4, 2, 128).transpose(2, 0, 3, 1, 4, 5)
    ).astype(bf)
    b1s = np.ascontiguousarray(np.asarray(b1, f).reshape(FC, 128).T)
    b2s = np.ascontiguousarray(np.asarray(b2, f).reshape(DC, 128).T)
    g1s = np.ascontiguousarray(np.asarray(ln1_g, f).reshape(DC, 128).T)
    e1s = np.ascontiguousarray(np.asarray(ln1_b, f).reshape(DC, 128).T)
    g2s = np.ascontiguousarray(np.asarray(ln2_g, f).reshape(DC, 128).T)
    e2s = np.ascontiguousarray(np.asarray(ln2_b, f).reshape(DC, 128).T)
    kk = np.arange(512)[:, None]
    qq = np.arange(512)[None, :]
    mask = (kk <= qq).astype(f).reshape(4, 128, 512).transpose(1, 0, 2)
    mask = np.ascontiguousarray(mask).astype(bf)

    in_maps = []
    for c in range(NCORES):
        b, j = c // 2, c % 2
        hb = j * HL
        xT = np.ascontiguousarray(
            x[b].T.reshape(DC, 128, S).transpose(1, 0, 2)).astype(bf)
        xtm = np.concatenate(
            [x[b, j * 512:(j + 1) * 512],
             x[b, (j + 2) * 512:(j + 3) * 512]]).T  # [D, TLOC]
        xtm = np.ascontiguousarray(
            xtm.reshape(DC, 128, TLOC).transpose(1, 0, 2)).astype(bf)
        wq = np.stack([np.concatenate(
            [Wq[hb + 2 * p], Wq[hb + 2 * p + 1]], 1) for p in range(NP)])
        wq = np.ascontiguousarray(
            (wq * SCALE).reshape(NP, DC, 128, 128).transpose(2, 0, 1, 3)
        ).astype(bf)
        wk = np.stack([np.concatenate(
            [Wk[hb + 2 * p], Wk[hb + 2 * p + 1]], 1) for p in range(NP)])
        wk = np.ascontiguousarray(
            wk.reshape(NP, DC, 128, 128).transpose(2, 0, 1, 3)).astype(bf)
        wv = np.concatenate([Wv[hb + i] for i in range(HL)], 1)  # [D, 512]
        wv = np.ascontiguousarray(
            wv.reshape(DC, 128, 512).transpose(1, 0, 2)).astype(bf)
        wo = Wo[j * 512:(j + 1) * 512]  # [512, D]
        wo = np.ascontiguousarray(
            wo.reshape(NP, 128, DC, 128).transpose(1, 0, 2, 3)).astype(bf)
        in_maps.append({
            "xT": xT, "xTm": xtm, "wq": wq, "wk": wk, "wv": wv, "wo": wo,
            "w1": w1p, "w2": w2p, "b1s": b1s, "b2s": b2s,
            "g1s": g1s, "e1s": e1s, "g2s": g2s, "e2s": e2s, "mask": mask,
        })
    return in_maps


def get_compiled():
    global _COMPILED
    if _COMPILED is None:
        _COMPILED = _build()
    return _COMPILED


def kernel(x, Wq, Wk, Wv, Wo, ln1_g, ln1_b, W1, b1, W2, b2, ln2_g, ln2_b,
           _trace=False):
    nc = get_compiled()
    in_maps = _pack_inputs(x, Wq, Wk, Wv, Wo, ln1_g, ln1_b, W1, b1, W2, b2,
                           ln2_g, ln2_b)
    res = run_bass_kernel_spmd(nc, in_maps, core_ids=list(range(NCORES)),
                               trace=_trace)
    out = np.zeros((B, S, D), np.float32)
    for c in range(NCORES):
        b, j = c // 2, c % 2
        o = res.results[c]["outT"]  # [128, DC, TLOC]
        o = o.transpose(1, 0, 2).reshape(D, TLOC)
        out[b, j * 512:(j + 1) * 512, :] = o[:, 0:512].T
        out[b, (j + 2) * 512:(j + 3) * 512, :] = o[:, 512:1024].T
    kernel.last_result = res
    return out


# revision 30
# speedup vs baseline: 1.0125x; 1.0125x over previous
"""Trainium2 Bass kernel for a single transformer decoder layer.

Sharding: 8 cores = 4 batches x 2 head-groups (tensor parallel over heads for
attention; pairwise ReduceScatter; token-split FFN). All activations are kept
feature-major ("transposed") on device; the host transposes inputs/outputs.
Token ownership after the reduce-scatter is by interleaved 512-blocks
({0,2} / {1,3}).

v2: full bf16 datapath (weights, activations, collective payloads), softmax
scale folded into Wq on the host, 2-bank PSUM score tiles with one wide exp,
approx reciprocals, normalize-broadcast via a tiny PE matmul, W1 resident in
SBUF so the FFN overlaps the second reduce-scatter.
"""

import contextlib
import sys

for _p in ("/opt/trn_rl_repo",):
    if _p not in sys.path:
        sys.path.insert(0, _p)

import numpy as np

import concourse.bass as bass
import concourse.mybir as mybir
import concourse.tile as tile
from concourse import bacc
from concourse.bass_utils import run_bass_kernel_spmd

# ---- problem constants (hardcoded per spec) ----
B, S, D = 4, 2048, 1024
H, DK, DV, DFF = 16, 64, 64, 4096
EPS = 1e-5
SCALE = 1.0 / 32.0  # 1/sqrt(D), folded into Wq on the host

NCORES = 8
HL = H // 2          # heads per core (local)
NP = HL // 2         # head-pairs per core (4)
TLOC = S // 2        # tokens owned per core after reduce-scatter (1024)
DC = D // 128        # d-model chunks (8)
FC = DFF // 128      # dff chunks (32)
FQ = FC // 4         # dff chunks per quarter (8)
QB = S // 512        # query blocks of 512 (4)

F32 = mybir.dt.float32
F32R = mybir.dt.float32r
BF16 = mybir.dt.bfloat16

DEBUG = False
_COMPILED = None


def _build():
    nc = bacc.Bacc("TRN2", target_bir_lowering=False, debug=False,
                   num_devices=NCORES)

    xT_d = nc.dram_tensor("xT", [128, DC, S], BF16, kind="ExternalInput").ap()
    xTm_d = nc.dram_tensor("xTm", [128, DC, TLOC], BF16,
                           kind="ExternalInput").ap()
    wq_d = nc.dram_tensor("wq", [128, NP, DC, 128], BF16,
                          kind="ExternalInput").ap()
    wk_d = nc.dram_tensor("wk", [128, NP, DC, 128], BF16,
                          kind="ExternalInput").ap()
    wv_d = nc.dram_tensor("wv", [128, DC, 512], BF16,
                          kind="ExternalInput").ap()
    wo_d = nc.dram_tensor("wo", [128, NP, DC, 128], BF16,
                          kind="ExternalInput").ap()
    w1_d = nc.dram_tensor("w1", [128, FC, DC, 128], BF16,
                          kind="ExternalInput").ap()
    w2_d = nc.dram_tensor("w2", [128, 4, 4, FQ, 2, 128], BF16,
                          kind="ExternalInput").ap()
    b1_d = nc.dram_tensor("b1s", [128, FC], F32, kind="ExternalInput").ap()
    b2_d = nc.dram_tensor("b2s", [128, DC], F32, kind="ExternalInput").ap()
    g1_d = nc.dram_tensor("g1s", [128, DC], F32, kind="ExternalInput").ap()
    e1_d = nc.dram_tensor("e1s", [128, DC], F32, kind="ExternalInput").ap()
    g2_d = nc.dram_tensor("g2s", [128, DC], F32, kind="ExternalInput").ap()
    e2_d = nc.dram_tensor("e2s", [128, DC], F32, kind="ExternalInput").ap()
    mk_d = nc.dram_tensor("mask", [128, 4, 512], BF16,
                          kind="ExternalInput").ap()

    outT_d = nc.dram_tensor("outT", [128, DC, TLOC], F32,
                            kind="ExternalOutput").ap()
    dbg = None
    if DEBUG:
        dbg = {
            "dbg_qt": nc.dram_tensor("dbg_qt", [128, NP, S], BF16,
                                     kind="ExternalOutput").ap(),
            "dbg_kt": nc.dram_tensor("dbg_kt", [128, NP, S], BF16,
                                     kind="ExternalOutput").ap(),
            "dbg_v": nc.dram_tensor("dbg_v", [128, S // 128, HL * 65], BF16,
                                    kind="ExternalOutput").ap(),
            "dbg_eab": nc.dram_tensor("dbg_eab", [128, 2, 512], BF16,
                                      kind="ExternalOutput").ap(),
            "dbg_ctx": nc.dram_tensor("dbg_ctx", [128, NP, 512], BF16,
                                      kind="ExternalOutput").ap(),
            "dbg_rsin": nc.dram_tensor("dbg_rsin", [2, D, 512], BF16,
                                       kind="ExternalOutput").ap(),
            "dbg_rso": nc.dram_tensor("dbg_rso", [D, 512], BF16,
                                      kind="ExternalOutput").ap(),
            "dbg_h1": nc.dram_tensor("dbg_h1", [128, DC, 512], BF16,
                                     kind="ExternalOutput").ap(),
            "dbg_o2": nc.dram_tensor("dbg_o2", [128, DC, 512], F32,
                                     kind="ExternalOutput").ap(),
        }

    with tile.TileContext(nc) as tc:
        _emit(nc, tc, xT_d, xTm_d, wq_d, wk_d, wv_d, wo_d, w1_d, w2_d,
              b1_d, b2_d, g1_d, e1_d, g2_d, e2_d, mk_d, outT_d, dbg)
    nc.compile()
    return nc


def _emit(nc, tc, xT_d, xTm_d, wq_d, wk_d, wv_d, wo_d, w1_d, w2_d,
          b1_d, b2_d, g1_d, e1_d, g2_d, e2_d, mk_d, outT_d, dbg=None):
    AF = mybir.ActivationFunctionType
    ALU = mybir.AluOpType

    with (
        tc.tile_pool(name="dram", bufs=1, space="DRAM") as dram,
        tc.tile_pool(name="const", bufs=1) as const,
        tc.tile_pool(name="pStA", bufs=2) as pStA,     # mu/rstd, 8KB
        tc.tile_pool(name="pStB", bufs=1) as pStB,     # var/srt, 4KB
        tc.tile_pool(name="pBC", bufs=2) as pBC,       # LN broadcasts, 4KB
        tc.tile_pool(name="pH1", bufs=1) as pH1,       # h1 bf16, 16KB
        tc.tile_pool(name="pDW", bufs=1) as pDW,       # d_half work, 24KB
    ):
        MASK = const.tile([128, 4, 512], BF16)
        # ones (value 1/D) for LN mean/var matmuls, f32r + bf16 flavors
        onesf = const.tile([128, 1], F32)
        nc.vector.memset(onesf[:], 1.0 / D)
        onesr = const.tile([128, 1], F32R)
        nc.vector.tensor_copy(onesr[:], onesf[:])
        onesb = const.tile([128, 1], BF16)
        nc.vector.tensor_copy(onesb[:], onesf[:])
        one1 = const.tile([128, 1], BF16)
        nc.vector.memset(one1[:], 1.0)
        # broadcast helper: ones64.T @ rec[1,512] -> [64,512]
        ones64 = const.tile([1, 64], BF16)
        nc.vector.memset(ones64[:], 1.0)
        epst = const.tile([1, 1], F32)
        nc.vector.memset(epst[:], EPS)
        g1t = const.tile([128, DC], F32)
        e1t = const.tile([128, DC], F32)
        g2t = const.tile([128, DC], F32)
        e2t = const.tile([128, DC], F32)
        b1t = const.tile([128, FC], F32)
        b2t = const.tile([128, DC], F32)

        def load_consts():
            nc.sync.dma_start(MASK[:], mk_d[:])
            for t_, d_ in ((g1t, g1_d), (e1t, e1_d), (g2t, g2_d),
                           (e2t, e2_d), (b1t, b1_d), (b2t, b2_d)):
                nc.sync.dma_start(t_[:], d_[:])

        rs_in0 = dram.tile([2, D, 512], BF16)
        rs_in1 = dram.tile([2, D, 512], BF16)
        rs_out0 = dram.tile([D, 512], BF16)
        rs_out1 = dram.tile([D, 512], BF16)

        def layer_norm_finish_stats(stat):
            """stat rows 0/64 hold sum*1/D and sumsq*1/D; -> (mu, rstd)."""
            mu = pStA.tile([1, 512], F32, tag="lnmu")
            nc.vector.tensor_copy(mu[:], stat[0:1, :])
            var = pStB.tile([1, 512], F32, tag="lnvar")
            nc.vector.tensor_mul(var[:], mu[:], mu[:])
            nc.vector.tensor_sub(var[:], stat[64:65, :], var[:])
            srt = pStB.tile([1, 512], F32, tag="lnsrt")
            nc.scalar.activation(srt[:], var[:], AF.Sqrt, bias=epst[:])
            rstd = pStA.tile([1, 512], F32, tag="lnrstd")
            nc.vector.reciprocal_approx_fast(rstd[:], srt[:])
            return mu, rstd

        def layer_norm_stats_f32r(src, sqb, psLN):
            """src: [128, DC, 512] f32r AP (properly rounded)."""
            nc.scalar.activation(sqb[:], src, AF.Square)
            stat = psLN.tile([128, 512], F32, tag="ctx")
            for dc in range(DC):
                nc.tensor.matmul(stat[0:1, :], onesr[:], src[:, dc],
                                 start=(dc == 0), stop=(dc == DC - 1))
            for dc in range(DC):
                nc.tensor.matmul(stat[64:65, :], onesb[:], sqb[:, dc],
                                 start=(dc == 0), stop=(dc == DC - 1))
            return layer_norm_finish_stats(stat)

        def layer_norm_apply(src, dst, mu, rstd, gt, et):
            """dst[:,dc] = ((src - mu) * rstd) * g + e; src modified in
            place. Broadcasts on gpsimd (safe: only used after the relevant
            collective in gpsimd queue order)."""
            mub = pBC.tile([128, 512], F32, tag="lnb")
            rstdb = pBC.tile([128, 512], F32, tag="lnb")
            nc.gpsimd.partition_broadcast(mub[:], mu[:])
            nc.gpsimd.partition_broadcast(rstdb[:], rstd[:])
            nc.vector.tensor_sub(
                src, src, mub[:, None, :].to_broadcast((128, DC, 512)))
            nc.vector.tensor_mul(
                src, src, rstdb[:, None, :].to_broadcast((128, DC, 512)))
            for dc in range(DC):
                nc.vector.tensor_scalar(
                    dst[:, dc], src[:, dc], gt[:, dc:dc + 1],
                    et[:, dc:dc + 1], ALU.mult, ALU.add)

        H1b = [pH1.tile([128, DC, 512], BF16, tag=f"H1b{h}",
                        name=f"H1b{h}") for h in range(2)]

        def d_half_dma(h, rso):
            """Kick off the DMAs for h1 = LN1(x + attn_out), half h."""
            aor = pDW.tile([128, DC, 512], BF16, tag="aor", name=f"aor{h}")
            nc.gpsimd.dma_start(
                aor[:], rso.rearrange("(dc p) t -> p dc t", p=128))
            xm = pDW.tile([128, DC, 512], BF16, tag="xm", name=f"xm{h}")
            nc.sync.dma_start(xm[:], xTm_d[:, :, h * 512:(h + 1) * 512])
            return aor, xm

        def d_half_prep(h, aor, xm, eng):
            """Elementwise prep; emit where `eng`'s queue has slack."""
            eng.tensor_add(xm[:], xm[:], aor[:])
            sq = pDW.tile([128, DC, 512], BF16, tag="sq", name=f"sq{h}")
            eng.tensor_mul(sq[:], xm[:], xm[:])
            return xm, xm, sq

        def d_half_finish(h, state, psLN, eng):
            """Stat matmuls + LN apply; emit where the PE has slack."""
            xm, xb, sq = state
            stat = psLN.tile([128, 512], F32, tag="ctx")
            for dc in range(DC):
                nc.tensor.matmul(stat[0:1, :], onesb[:], xb[:, dc],
                                 start=(dc == 0), stop=(dc == DC - 1))
            for dc in range(DC):
                nc.tensor.matmul(stat[64:65, :], onesb[:], sq[:, dc],
                                 start=(dc == 0), stop=(dc == DC - 1))
            mu, rstd = layer_norm_finish_stats(stat)
            mub = pBC.tile([128, 512], F32, tag="lnb")
            rstdb = pBC.tile([128, 512], F32, tag="lnb")
            nc.gpsimd.partition_broadcast(mub[:], mu[:])
            nc.gpsimd.partition_broadcast(rstdb[:], rstd[:])
            eng.tensor_sub(
                xm[:], xm[:], mub[:, None, :].to_broadcast((128, DC, 512)))
            eng.tensor_mul(
                xm[:], xm[:], rstdb[:, None, :].to_broadcast((128, DC, 512)))
            for dc in range(DC):
                eng.tensor_scalar(
                    H1b[h][:, dc], xm[:, dc], g1t[:, dc:dc + 1],
                    e1t[:, dc:dc + 1], ALU.mult, ALU.add)

        w1stack = contextlib.ExitStack()
        with (
            tc.tile_pool(name="pQKT", bufs=1) as pQKT,
            tc.tile_pool(name="pV", bufs=1) as pV,
        ):
            QT = pQKT.tile([128, NP, S], BF16, tag="QT")
            KT = pQKT.tile([128, NP, S], BF16, tag="KT")
            V = pV.tile([128, S // 128, HL * 65], BF16, tag="V")
            nc.vector.tensor_copy(
                V[:].rearrange("p t (h c) -> p t h c", c=65)[:, :, :, 64:65],
                one1[:, None, None, :].to_broadcast((128, S // 128, HL, 1)))

            # ================= Q/K/V projections =================
            with (
                tc.tile_pool(name="pX", bufs=1) as pX,
                tc.tile_pool(name="pWqkv", bufs=1) as pWqkv,
            ):
                wqt = pWqkv.tile([128, NP, DC, 128], BF16, tag="wq")
                wkt = pWqkv.tile([128, NP, DC, 128], BF16, tag="wk")
                wvt = pWqkv.tile([128, DC, 512], BF16, tag="wv")
                nc.sync.dma_start(wqt[:], wq_d[:])
                nc.sync.dma_start(wkt[:], wk_d[:])
                X = pX.tile([128, DC, S], BF16, tag="X")
                for dc in range(DC):
                    nc.sync.dma_start(X[:, dc], xT_d[:, dc])
                nc.sync.dma_start(wvt[:], wv_d[:])
                load_consts()

                with tc.tile_pool(name="psP", bufs=8, space="PSUM") as psP:
                    for p in range(NP):
                        pqs = [psP.tile([128, 512], F32, tag="proj",
                                        name=f"pq_{p}_{i}") for i in range(8)]
                        for dc in range(DC):
                            for tb in range(QB):
                                nc.tensor.matmul(
                                    pqs[tb][:], wqt[:, p, dc],
                                    X[:, dc, tb * 512:(tb + 1) * 512],
                                    start=(dc == 0), stop=(dc == DC - 1))
                            for tb in range(QB):
                                nc.tensor.matmul(
                                    pqs[4 + tb][:], wkt[:, p, dc],
                                    X[:, dc, tb * 512:(tb + 1) * 512],
                                    start=(dc == 0), stop=(dc == DC - 1))
                        for tb in range(QB):
                            tsl = slice(tb * 512, (tb + 1) * 512)
                            nc.scalar.activation(QT[:, p, tsl], pqs[tb][:],
                                                 AF.Identity)
                            nc.vector.tensor_copy(KT[:, p, tsl],
                                                  pqs[4 + tb][:])

                with tc.tile_pool(name="psV", bufs=3, space="PSUM") as psV:
                    for tt in range(S // 128):
                        pv = psV.tile([128, 512], F32, tag="pv")
                        for dc in range(DC):
                            nc.tensor.matmul(
                                pv[:], X[:, dc, tt * 128:(tt + 1) * 128],
                                wvt[:, dc],
                                start=(dc == 0), stop=(dc == DC - 1))
                        nc.vector.tensor_copy(
                            V[:, tt].rearrange("p (h c) -> p h c",
                                               c=65)[:, :, 0:64],
                            pv[:].rearrange("p (h c) -> p h c", c=64))

            if dbg is not None:
                nc.sync.dma_start(dbg["dbg_qt"][:], QT[:])
                nc.sync.dma_start(dbg["dbg_kt"][:], KT[:])
                nc.sync.dma_start(dbg["dbg_v"][:], V[:])

            # W1 stays resident through the FFN (ExitStack outlives this
            # block); loaded during attention via the freed X/Wqkv space.
            pW1 = w1stack.enter_context(tc.tile_pool(name="pW1", bufs=1,
                                                     side="right"))
            w1t = pW1.tile([128, FC, DC, 128], BF16, tag="w1")
            for fq in range(4):
                nc.sync.dma_start(w1t[:, fq * FQ:(fq + 1) * FQ],
                                  w1_d[:, fq * FQ:(fq + 1) * FQ])

            # ==================== attention ====================
            with (
                tc.tile_pool(name="pWO", bufs=1) as pWO,
                tc.tile_pool(name="pCTX", bufs=1) as pCTX,
                tc.tile_pool(name="pE", bufs=2) as pE,
                tc.tile_pool(name="pAO", bufs=2) as pAO,
                tc.tile_pool(name="pRec", bufs=1) as pRec,
                tc.tile_pool(name="psS", bufs=3, space="PSUM") as psS,
                tc.tile_pool(name="psC", bufs=2, space="PSUM") as psC,
            ):
                wot = pWO.tile([128, NP, DC, 128], BF16, tag="wo")
                nc.sync.dma_start(wot[:], wo_d[:])

                pend = [None]
                d0_dma = [None]
                d0_state = [None]

                def flush_norm():
                    if pend[0] is None:
                        return
                    ctx_t, pp, rec16 = pend[0]
                    pend[0] = None
                    rb = psS.tile([128, 2, 512], F32, tag="sc")
                    recb = rb[:, 0]
                    nc.tensor.matmul(recb[0:64, :], ones64[:],
                                     rec16[0:1, 0:512],
                                     start=True, stop=True)
                    nc.tensor.matmul(recb[64:128, :], ones64[:],
                                     rec16[0:1, 512:1024],
                                     start=True, stop=True)
                    nc.vector.tensor_mul(ctx_t[:, pp, :], ctx_t[:, pp, :],
                                         recb[:])

                for qb in range(QB):
                    qsl = slice(qb * 512, (qb + 1) * 512)
                    nkc = 4 * (qb + 1)
                    CTX = pCTX.tile([128, NP, 512], BF16, tag="CTX")
                    for p in range(NP):
                        ctxA = psC.tile([128, 512], F32, tag="ctx")
                        ctxB = psC.tile([128, 512], F32, tag="ctx")
                        for kc in range(nkc):
                            ksl = slice(kc * 128, (kc + 1) * 128)
                            # columns below the diagonal block start are
                            # fully masked: skip them in scores/exp/ctx
                            mkc = kc - 4 * qb
                            off = max(mkc, 0) * 128
                            lsl = slice(off, 512)
                            qss = slice(qb * 512 + off, (qb + 1) * 512)
                            ps = psS.tile([128, 2, 512], F32, tag="sc")
                            nc.tensor.matmul(ps[:, 0, lsl],
                                             KT[0:64, p, ksl],
                                             QT[0:64, p, qss],
                                             start=True, stop=True)
                            nc.tensor.matmul(ps[:, 1, lsl],
                                             KT[64:128, p, ksl],
                                             QT[64:128, p, qss],
                                             start=True, stop=True)
                            eAB = pE.tile([128, 2, 512], BF16, tag="E")
                            nc.scalar.activation(eAB[:, :, lsl],
                                                 ps[:, :, lsl], AF.Exp)
                            if mkc >= 0:
                                nc.vector.tensor_mul(
                                    eAB[:, :, lsl], eAB[:, :, lsl],
                                    MASK[:, mkc, lsl][:, None, :]
                                    .to_broadcast((128, 2, 512 - off)))
                            if dbg is not None and qb == 0 and p == 0 \
                                    and kc == 0:
                                nc.sync.dma_start(dbg["dbg_eab"][:], eAB[:])
                            st, sp = (kc == 0), (kc == nkc - 1)
                            nc.tensor.matmul(
                                ctxA[0:65, lsl],
                                V[:, kc, 2 * p * 65:(2 * p + 1) * 65],
                                eAB[:, 0, lsl], start=st, stop=sp)
                            nc.tensor.matmul(
                                ctxB[0:65, lsl],
                                V[:, kc, (2 * p + 1) * 65:(2 * p + 2) * 65],
                                eAB[:, 1, lsl], start=st, stop=sp)
                        # normalize: rec = 1/rowsum; per-head PE broadcast
                        den = pRec.tile([1, 1024], F32, tag="den")
                        nc.vector.tensor_copy(den[0:1, 0:512],
                                              ctxA[64:65, :])
                        nc.vector.tensor_copy(den[0:1, 512:1024],
                                              ctxB[64:65, :])
                        recF = pRec.tile([1, 1024], F32, tag="recF")
                        rec16 = pRec.tile([1, 1024], BF16, tag="rec16")
                        nc.vector.reciprocal_approx_fast(recF[:], den[:])
                        nc.vector.tensor_copy(rec16[:], recF[:])
                        nc.vector.tensor_copy(CTX[0:64, p, :], ctxA[0:64, :])
                        nc.vector.tensor_copy(CTX[64:128, p, :],
                                              ctxB[0:64, :])
                        flush_norm()
                        pend[0] = (CTX, p, rec16)
                        if qb == 3 and p == 0:
                            d0_state[0] = d_half_prep(0, *d0_dma[0],
                                                      nc.gpsimd)
                        if qb == 3 and p == 2:
                            d_half_finish(0, d0_state[0], psC, nc.gpsimd)
                    flush_norm()
                    if dbg is not None and qb == 0:
                        nc.sync.dma_start(dbg["dbg_ctx"][:], CTX[:])
                    # Wo partial for this token block
                    rsdst = rs_in0 if qb < 2 else rs_in1
                    for dout in range(DC):
                        po = psC.tile([128, 512], F32, tag="ctx")
                        for p in range(NP):
                            nc.tensor.matmul(po[:], wot[:, p, dout],
                                             CTX[:, p, :],
                                             start=(p == 0),
                                             stop=(p == NP - 1))
                        ao = pAO.tile([128, 512], BF16, tag="ao")
                        nc.vector.tensor_copy(ao[:], po[:])
                        nc.sync.dma_start(
                            rsdst[qb % 2, dout * 128:(dout + 1) * 128, :],
                            ao[:])
                    if qb == 1:
                        nc.gpsimd.collective_compute(
                            "ReduceScatter", mybir.AluOpType.add,
                            replica_groups=[[0, 1], [2, 3], [4, 5], [6, 7]],
                            ins=[rs_in0.opt()], outs=[rs_out0.opt()])
                        if dbg is not None:
                            nc.gpsimd.dma_start(dbg["dbg_rsin"][:],
                                                rs_in0[:])
                            nc.gpsimd.dma_start(dbg["dbg_rso"][:],
                                                rs_out0[:])
                        d0_dma[0] = d_half_dma(0, rs_out0)

                nc.gpsimd.collective_compute(
                    "ReduceScatter", mybir.AluOpType.add,
                    replica_groups=[[0, 1], [2, 3], [4, 5], [6, 7]],
                    ins=[rs_in1.opt()], outs=[rs_out1.opt()])

        # ======== FFN (W1 resident, W2 streamed) ========
        with (
            tc.tile_pool(name="pFF", bufs=1) as pFF,
            tc.tile_pool(name="pO2", bufs=1) as pO2,
            tc.tile_pool(name="pW2q", bufs=3) as pW2q,
            tc.tile_pool(name="pOT", bufs=2) as pOT,
            tc.tile_pool(name="psF", bufs=2, space="PSUM") as psF,
            tc.tile_pool(name="psG", bufs=4, space="PSUM") as psG,
            tc.tile_pool(name="psL", bufs=2, space="PSUM") as psL,
        ):
            FFt = pFF.tile([128, FQ, 512], BF16, tag="FF")
            O2h = [pO2.tile([128, DC, 512], F32R, tag=f"O2_{h}",
                            name=f"O2_{h}") for h in range(2)]

            def ffn_half(th, hooks=None):
                for fq in range(4):
                    if hooks and fq in hooks:
                        hooks[fq]()
                    for fi in range(FQ):
                        fc = fq * FQ + fi
                        pf = psF.tile([128, 512], F32, tag="ff")
                        for dc in range(DC):
                            nc.tensor.matmul(
                                pf[:], w1t[:, fc, dc], H1b[th][:, dc],
                                start=(dc == 0), stop=(dc == DC - 1))
                        nc.scalar.activation(FFt[:, fi], pf[:], AF.Relu,
                                             bias=b1t[:, fc:fc + 1])
                    if hooks and ("w1_" + str(fq)) in hooks:
                        hooks["w1_" + str(fq)]()
                    for dq in range(4):
                        w2q = pW2q.tile([128, FQ, 2, 128], BF16, tag="w2")
                        nc.sync.dma_start(w2q[:], w2_d[:, fq, dq])
                        pos = [psG.tile([128, 512], F32, tag="o2",
                                        name=f"o2_{i}") for i in range(2)]
                        for fi in range(FQ):
                            for do2 in range(2):
                                nc.tensor.matmul(
                                    pos[do2][:], w2q[:, fi, do2], FFt[:, fi],
                                    start=(fi == 0), stop=(fi == FQ - 1))
                        for do2 in range(2):
                            dout = dq * 2 + do2
                            if fq == 0:
                                # O2 = W2 partial + b2 (bias folded in here)
                                nc.vector.tensor_scalar_add(
                                    O2h[th][:, dout], pos[do2][:],
                                    b2t[:, dout:dout + 1])
                            else:
                                nc.vector.tensor_add(
                                    O2h[th][:, dout], O2h[th][:, dout],
                                    pos[do2][:])
                            if fq == 3:
                                # final quarter: fold in the h1 residual
                                nc.vector.tensor_add(
                                    O2h[th][:, dout], O2h[th][:, dout],
                                    H1b[th][:, dout])

            def finish_half(th):
                # O2h already holds ff + h1 + b2; LN2 + store, per-dc
                if dbg is not None and th == 0:
                    nc.sync.dma_start(dbg["dbg_h1"][:], H1b[0][:])
                    nc.sync.dma_start(dbg["dbg_o2"][:],
                                      O2h[0][:].bitcast(F32))
                sqb = pFF.tile([128, DC, 512], BF16, tag="fsq",
                               name=f"fsq{th}")
                mu, rstd = layer_norm_stats_f32r(O2h[th][:], sqb, psL)
                mub = pBC.tile([128, 512], F32, tag="lnb")
                rstdb = pBC.tile([128, 512], F32, tag="lnb")
                nc.gpsimd.partition_broadcast(mub[:], mu[:])
                nc.gpsimd.partition_broadcast(rstdb[:], rstd[:])
                for dc in range(DC):
                    nc.vector.tensor_sub(O2h[th][:, dc], O2h[th][:, dc],
                                         mub[:])
                    nc.vector.tensor_mul(O2h[th][:, dc], O2h[th][:, dc],
                                         rstdb[:])
                    ot = pOT.tile([128, 512], F32, tag="ot")
                    nc.vector.tensor_scalar(
                        ot[:], O2h[th][:, dc], g2t[:, dc:dc + 1],
                        e2t[:, dc:dc + 1], ALU.mult, ALU.add)
                    nc.sync.dma_start(
                        outT_d[:, dc, th * 512:(th + 1) * 512], ot[:])

            d1_dma = d_half_dma(1, rs_out1)
            d1_state = [None]

            def d1_prep():
                d1_state[0] = d_half_prep(1, *d1_dma, nc.gpsimd)

            def d1_finish():
                d_half_finish(1, d1_state[0], psL, nc.gpsimd)

            ffn_half(0, hooks={2: d1_prep, "w1_3": d1_finish})
            finish_half(0)
            ffn_half(1)
            finish_half(1)
        w1stack.close()


def _pack_inputs(x, Wq, Wk, Wv, Wo, ln1_g, ln1_b, W1, b1, W2, b2, ln2_g,
                 ln2_b):
    """Build the 8 per-core input maps (all host-side numpy)."""
    import ml_dtypes
    bf = ml_dtypes.bfloat16
    f = np.float32
    x = np.asarray(x, f)
    Wq = np.asarray(Wq, f); Wk = np.asarray(Wk, f); Wv = np.asarray(Wv, f)
    Wo = np.asarray(Wo, f); W1 = np.asarray(W1, f); W2 = np.asarray(W2, f)

    w1p = np.ascontiguousarray(
        W1.reshape(DC, 128, FC, 128).transpose(1, 2, 0, 3)).astype(bf)
    w2p = np.ascontiguousarray(
        W2.reshape(4, FQ, 128, 4, 2, 128).transpose(2, 0, 3, 1, 4, 5)
    ).astype(bf)
    b1s = np.ascontiguousarray(np.asarray(b1, f).reshape(FC, 128).T)
    b2s = np.ascontiguousarray(np.asarray(b2, f).reshape(DC, 128).T)
    g1s = np.ascontiguousarray(np.asarray(ln1_g, f).reshape(DC, 128).T)
    e1s = np.ascontiguousarray(np.asarray(ln1_b, f).reshape(DC, 128).T)
    g2s = np.ascontiguousarray(np.asarray(ln2_g, f).reshape(DC, 128).T)
    e2s = np.ascontiguousarray(np.asarray(ln2_b, f).reshape(DC, 128).T)
    kk = np.arange(512)[:, None]
    qq = np.arange(512)[None, :]
    mask = (kk <= qq).astype(f).reshape(4, 128, 512).transpose(1, 0, 2)
    mask = np.ascontiguousarray(mask).astype(bf)

    in_maps = []
    for c in range(NCORES):
        b, j = c // 2, c % 2
        hb = j * HL
        xT = np.ascontiguousarray(
            x[b].T.reshape(DC, 128, S).transpose(1, 0, 2)).astype(bf)
        xtm = np.concatenate(
            [x[b, j * 512:(j + 1) * 512],
             x[b, (j + 2) * 512:(j + 3) * 512]]).T  # [D, TLOC]
        xtm = np.ascontiguousarray(
            xtm.reshape(DC, 128, TLOC).transpose(1, 0, 2)).astype(bf)
        wq = np.stack([np.concatenate(
            [Wq[hb + 2 * p], Wq[hb + 2 * p + 1]], 1) for p in range(NP)])
        wq = np.ascontiguousarray(
            (wq * SCALE).reshape(NP, DC, 128, 128).transpose(2, 0, 1, 3)
        ).astype(bf)
        wk = np.stack([np.concatenate(
            [Wk[hb + 2 * p], Wk[hb + 2 * p + 1]], 1) for p in range(NP)])
        wk = np.ascontiguousarray(
            wk.reshape(NP, DC, 128, 128).transpose(2, 0, 1, 3)).astype(bf)
        wv = np.concatenate([Wv[hb + i] for i in range(HL)], 1)  # [D, 512]
        wv = np.ascontiguousarray(
            wv.reshape(DC, 128, 512).transpose(1, 0, 2)).astype(bf)
        wo = Wo[j * 512:(j + 1) * 512]  # [512, D]
        wo = np.ascontiguousarray(
            wo.reshape(NP, 128, DC, 128).transpose(1, 0, 2, 3)).astype(bf)
        in_maps.append({
            "xT": xT, "xTm": xtm, "wq": wq, "wk": wk, "wv": wv, "wo": wo,
            "w1": w1p, "w2": w2p, "b1s": b1s, "b2s": b2s,
            "g1s": g1s, "e1s": e1s, "g2s": g2s, "e2s": e2s, "mask": mask,
        })
    return in_maps


def get_compiled():
    global _COMPILED
    if _COMPILED is None:
        _COMPILED = _build()
    return _COMPILED


def kernel(x, Wq, Wk, Wv, Wo, ln1_g, ln1_b, W1, b1, W2, b2, ln2_g, ln2_b,
           _trace=False):
    nc = get_compiled()
    in_maps = _pack_inputs(x, Wq, Wk, Wv, Wo, ln1_g, ln1_b, W1, b1, W2, b2,
                           ln2_g, ln2_b)
    res = run_bass_kernel_spmd(nc, in_maps, core_ids=list(range(NCORES)),
                               trace=_trace)
    out = np.zeros((B, S, D), np.float32)
    for c in range(NCORES):
        b, j = c // 2, c % 2
        o = res.results[c]["outT"]  # [128, DC, TLOC]
        o = o.transpose(1, 0, 2).reshape(D, TLOC)
        out[b, j * 512:(j + 1) * 512, :] = o[:, 0:512].T
        out[b, (j + 2) * 512:(j + 3) * 512, :] = o[:, 512:1024].T
    kernel.last_result = res
    return out
